# revision 39
# baseline (speedup 1.0000x reference)
"""Bidirectional ConvLSTM + 1x1 proj + BatchNorm + ReLU + skip, on 8 trn2 cores.

Sharding: data-parallel over batch (B=8 -> 1 batch element per core).
BatchNorm batch statistics are reduced across cores with a tiny AllReduce.

Per-core layout: channels on SBUF partitions (96), W on the free dim.
The H-recurrence runs forward and backward interleaved (192 slots x 2 dirs).
Gate banks: ps0=[f|i], ps1=[2g|o]; g-rows are pre-doubled so ONE fused
sigmoid per bank covers all gates (tanh g = 2*sigmoid(2g)-1, folded into
scalar_tensor_tensor ops downstream).

The 1x1 projection (pass A) runs inside the scan, lagged so both dirs'
h-rows exist; its result y is copied to a bf16 SBUF store, so the old
recompute pass B is gone: the tail is just AllReduce + relu/skip/store.
h-stores are half-persistent (96 slots for the early rows a dir writes,
which the other dir needs up to ~H steps later) + a 4-deep rolling window
for the fresh rows.
"""

import os
import sys
import types

import numpy as np
import ml_dtypes

B, C, H, W = 8, 96, 192, 192
HC = 96
EPS = 1e-5
NCORES = 8
WP = W + 2          # padded row width in the hidden-state store
NROLL = 4           # rolling slots for fresh rows
NSLOT = 96 + NROLL + 1  # 96 persistent + rolling + one zero slot
ZSLOT = NSLOT - 1
HWTOT = H * W       # 36864
NSTAT = 96          # bn-stats chunks (2 rows each)

_cached = {}


def _slot(t):
    # store slot written by a dir at its step t
    return t if t < 96 else 96 + (t % NROLL)


def _install_ntff_hook():
    # Optional: lets BASS_TRACE=1 produce an NTFF profile under axon.
    if 'antenv.axon_hooks' in sys.modules:
        return
    try:
        import trn_agent_boot.trn_boot as tb
        hook = tb._ntff_profile_via_ctypes('/opt/axon/libaxon_pjrt.so')
        mod = types.ModuleType('antenv.axon_hooks')
        mod.get_axon_ntff_profile_hook = lambda: hook
        mod.set_axon_ntff_profile_hook = lambda h: None
        sys.modules['antenv.axon_hooks'] = mod
    except Exception:
        pass


def _prep_weights(w_f, b_f, w_b, b_b, w_proj):
    """Host-side weight packing.

    Gate bank layout: bank0 = [f | i], bank1 = [2g | o]; gb order (f,i,g,o);
    g rows (incl bias) doubled (tanh g = 2*sigmoid(2g)-1 downstream).
    Reference gate row ranges: i=0:96, f=96:192, g=192:288, o=288:384.

    Returns: wx [97,2,12,96] bf16 (k=channel(+bias ones-row 96), j=gb*3+tap),
             wh [96,2,12,96] bf16, wp [96,2,96] bf16.
    """
    bf16 = ml_dtypes.bfloat16
    gate_rows = [slice(96, 192), slice(0, 96), slice(192, 288), slice(288, 384)]
    gate_scale = [1.0, 1.0, 2.0, 1.0]
    # raw per-(d, gb): rx [96k, 3tap, 96m], rh [96k, 3tap, 96m], rb [96m]
    rx = np.zeros((2, 4, 96, 3, 96), np.float32)
    rh = np.zeros((2, 4, 96, 3, 96), np.float32)
    rb = np.zeros((2, 4, 96), np.float32)
    for d, (w4, bias) in enumerate(((w_f, b_f), (w_b, b_b))):
        wmid = w4[:, :, 1, :]          # [384, 192, 3]
        for gb in range(4):
            rows = gate_rows[gb]
            g = gate_scale[gb]
            for tap in range(3):
                rx[d, gb, :, tap, :] = g * wmid[rows, 0:96, tap].T
                rh[d, gb, :, tap, :] = g * wmid[rows, 96:192, tap].T
            rb[d, gb, :] = g * bias[rows]
    # all-bf16 gate weights: fp8 (e4m3) was tried for the conv taps via
    # DoubleRow pairs but its ~3.6% systematic weight error accumulates
    # coherently through the recurrence (1/(1-f) amplification) and lands
    # at ~2e-2 output error -- over budget. bf16 stays.
    wx = np.zeros((97, 2, 12, 96), np.float32)
    wh = np.zeros((96, 2, 12, 96), np.float32)
    for d in range(2):
        for gb in range(4):
            for tap in range(3):
                j = gb * 3 + tap
                wx[0:96, d, j, :] = rx[d, gb, :, tap, :]
                wh[:, d, j, :] = rh[d, gb, :, tap, :]
            wx[96, d, gb * 3 + 1, :] = rb[d, gb, :]
    wp = np.zeros((96, 2, 96), np.float32)
    wp[:, 0, :] = w_proj[:, 0:96].T
    wp[:, 1, :] = w_proj[:, 96:192].T
    return wx.astype(bf16), wh.astype(bf16), wp.astype(bf16)


def _build_program():
    import concourse.bass as bass
    import concourse.bacc as bacc
    import concourse.tile as tile
    from concourse import mybir

    import bass_rust
    DR = bass_rust.MatmulPerfMode.DoubleRow
    f32 = mybir.dt.float32
    bf16 = mybir.dt.bfloat16
    fp8 = mybir.dt.float8e4
    AF = mybir.ActivationFunctionType
    ALU = mybir.AluOpType

    nc = bacc.Bacc('TRN2', target_bir_lowering=False, debug=False,
                   num_devices=NCORES)

    xbf_d = nc.dram_tensor("xbf", [C, HWTOT], bf16, kind="ExternalInput")
    wx_d = nc.dram_tensor("wx", [97, 2, 12, 96], bf16, kind="ExternalInput")
    wh_d = nc.dram_tensor("wh", [96, 2, 12, 96], bf16, kind="ExternalInput")
    wp_d = nc.dram_tensor("wp", [96, 2, 96], bf16, kind="ExternalInput")
    gb_d = nc.dram_tensor("gamma_beta", [96, 2], f32, kind="ExternalInput")
    out_d = nc.dram_tensor("out", [C, HWTOT], f32, kind="ExternalOutput")

    with tile.TileContext(nc) as tc:
        with (
            tc.tile_pool(name="const", bufs=1) as const,
            tc.tile_pool(name="dram", bufs=1, space="DRAM") as dram,
        ):
            # --- constants / persistent state -------------------------------
            wx_s = const.tile([97, 2, 12, 96], bf16, name="wx_s")
            nc.gpsimd.dma_start(wx_s[:], wx_d[:])
            wh_s = const.tile([96, 2, 12, 96], bf16, name="wh_s")
            nc.gpsimd.dma_start(wh_s[:], wh_d[:])
            wp_s = const.tile([96, 2, 96], bf16, name="wp_s")
            nc.gpsimd.dma_start(wp_s[:], wp_d[:])
            gb_s = const.tile([96, 2], f32, name="gb_s")
            nc.gpsimd.dma_start(gb_s[:], gb_d[:])
            eps_s = const.tile([96, 1], f32, name="eps_s")
            nc.vector.memset(eps_s[:], EPS)

            # warmup AllReduce: absorb the CC-engine bootstrap latency while
            # the scan runs, so the real stats AllReduce at the end is fast
            wu_i = dram.tile([96, 2], f32, name="wu_i")
            wu_o = dram.tile([96, 2], f32, name="wu_o")
            wu_s = const.tile([96, 2], f32, name="wu_s")
            nc.vector.memset(wu_s[:], 0.0)
            nc.gpsimd.dma_start(wu_i[:], wu_s[:])
            nc.gpsimd.collective_compute(
                "AllReduce",
                bass.mybir.AluOpType.add,
                replica_groups=[list(range(NCORES))],
                ins=[wu_i.opt()],
                outs=[wu_o.opt()],
            )

            # projected-y store (bf16), written during the scan, row-indexed
            y_s = const.tile([96, H, W], bf16, name="y_s")

            statb = const.tile([96, NSTAT, 6], f32, name="statb")

            # --- the scan ---------------------------------------------------
            with (
                tc.tile_pool(name="scanbuf", bufs=1) as scanbuf,
                tc.tile_pool(name="sact", bufs=2) as sact,
                tc.tile_pool(name="tmp", bufs=2) as tmp,
                tc.tile_pool(name="gpsum", bufs=2, space="PSUM") as gpsum,
            ):
                # hidden-state stores: 96 persistent + rolling + zero slot
                hs = []
                for d in range(2):
                    st = scanbuf.tile([96, NSLOT, WP], bf16, name=f"hs{d}")
                    nc.vector.memset(st[:, :, 0:1], 0.0)
                    nc.vector.memset(st[:, :, WP - 1:WP], 0.0)
                    nc.vector.memset(st[:, ZSLOT, :], 0.0)
                    hs.append(st)

                # x-row tiles (96 ch + ones row), 3-deep rotation per dir
                XRDEPTH = 3
                xr = [[None] * XRDEPTH, [None] * XRDEPTH]
                for d in range(2):
                    for p in range(XRDEPTH):
                        t = scanbuf.tile([97, WP], bf16, name=f"xr{d}{p}")
                        nc.vector.memset(t[0:96, 0:1], 0.0)
                        nc.vector.memset(t[0:96, WP - 1:WP], 0.0)
                        nc.vector.memset(t[96:97, :], 1.0)
                        xr[d][p] = t



                # cell state fp32, ping-pong per dir
                ctl = [[None, None], [None, None]]
                for d in range(2):
                    for p in range(2):
                        t = scanbuf.tile([96, W], f32, name=f"c{d}{p}")
                        nc.vector.memset(t[:], 0.0)
                        ctl[d][p] = t
                def pass_a(t):
                    # project rows (ra, rb); both dirs' h exist by now.
                    ra, rb = t - 2, 193 - t
                    k = t - 98
                    psA = gpsum.tile([96, 512], f32, name="psA")
                    # y[ra]: h_f fresh (rolling), h_b old (persistent)
                    nc.tensor.matmul(psA[:, 0:192], wp_s[:, 0, :],
                                     hs[0][:, _slot(ra), 1:1 + W],
                                     start=True, stop=False)
                    nc.tensor.matmul(psA[:, 0:192], wp_s[:, 1, :],
                                     hs[1][:, 191 - ra, 1:1 + W],
                                     start=False, stop=False)
                    # y[rb]: h_f old (persistent), h_b fresh (rolling)
                    nc.tensor.matmul(psA[:, 192:384], wp_s[:, 0, :],
                                     hs[0][:, rb, 1:1 + W],
                                     start=False, stop=False)
                    nc.tensor.matmul(psA[:, 192:384], wp_s[:, 1, :],
                                     hs[1][:, _slot(t - 2), 1:1 + W],
                                     start=False, stop=True)
                    nc.vector.bn_stats(statb[:, k, :], psA[:, 0:384])
                    nc.vector.tensor_copy(y_s[:, ra, :], psA[:, 0:192])
                    nc.vector.tensor_copy(y_s[:, rb, :], psA[:, 192:384])

                for t in range(H):
                    px = t % XRDEPTH
                    p, pn = t & 1, (t + 1) & 1
                    if t >= 98:
                        pass_a(t)

                    for d in range(2):
                        r = t if d == 0 else H - 1 - t
                        sl_in = ZSLOT if t == 0 else _slot(t - 1)
                        sl_out = _slot(t)

                        nc.gpsimd.dma_start(xr[d][px][0:96, 1:1 + W],
                                            xbf_d[:, r * W:(r + 1) * W])

                        # bank0 = [f|i], bank1 = [2g|o]; per bank all x taps
                        # first, then h taps (widens the h2-wait margin).
                        ps0 = gpsum.tile([96, 512], f32, name=f"ps0_{d}", bufs=1)
                        ps1 = gpsum.tile([96, 512], f32, name=f"ps1_{d}")
                        hin = hs[d][:, sl_in, :]
                        sa = [None, None]
                        for bank, (pst, gb0, gb1) in enumerate(
                            ((ps0, 0, 1), (ps1, 2, 3))
                        ):
                            for gbi, col in ((gb0, 0), (gb1, 192)):
                                for tap in range(3):
                                    j = gbi * 3 + tap
                                    nc.tensor.matmul(
                                        pst[:, col:col + 192],
                                        wx_s[:, d, j, :],
                                        xr[d][px][:, tap:tap + W],
                                        start=(col == 0 and tap == 0),
                                        stop=False,
                                    )
                            for gbi, col in ((gb0, 0), (gb1, 192)):
                                for tap in range(3):
                                    j = gbi * 3 + tap
                                    nc.tensor.matmul(
                                        pst[:, col:col + 192],
                                        wh_s[:, d, j, :],
                                        hin[:, tap:tap + W],
                                        start=False,
                                        stop=(col == 192 and tap == 2),
                                    )
                            # one fused sigmoid per bank
                            sa[bank] = sact.tile([96, 384], bf16,
                                                 name=f"sa{bank}_{d}")
                            nc.scalar.activation(sa[bank][:], pst[:, 0:384],
                                                 AF.Sigmoid)
                            if bank == 0:
                                # t1 = sig(f) * c_prev, off critical path
                                t1 = tmp.tile([96, W], f32, name=f"t1_{d}")
                                nc.vector.tensor_mul(t1[:], sa[0][:, 0:192],
                                                     ctl[d][p][:])

                        sf_i = sa[0]   # [f|i]
                        sg_o = sa[1]   # [sig(2g)|o]
                        # t2' = (sig(2g) - 0.5) * sig(i)  ( = tanh(g)*sig(i)/2 )
                        t2p = tmp.tile([96, W], bf16, name=f"t2p_{d}")
                        nc.vector.scalar_tensor_tensor(
                            t2p[:], sg_o[:, 0:192], 0.5, sf_i[:, 192:384],
                            ALU.subtract, ALU.mult)
                        # c2 = 2*t2' + t1
                        nc.vector.scalar_tensor_tensor(
                            ctl[d][pn][:], t2p[:], 2.0, t1[:],
                            ALU.mult, ALU.add)
                        tc2 = tmp.tile([96, W], bf16, name=f"tc2_{d}")
                        nc.scalar.activation(tc2[:], ctl[d][pn][:], AF.Tanh)
                        # h2 -> store (bf16), serves recurrence + projection
                        nc.vector.tensor_mul(hs[d][:, sl_out, 1:1 + W],
                                             sg_o[:, 192:384], tc2[:])

                # flush the remaining projection rows (ra=190,191 / rb=1,0)
                for t in (H, H + 1):
                    pass_a(t)

                mv = const.tile([96, 2], f32, name="mv")
                nc.vector.bn_aggr(mv[:], statb[:])
                # partial sums: s1 = mean*n, s2 = (var + mean^2)*n
                n_core = float(HWTOT)
                msq = const.tile([96, 1], f32, name="msq")
                nc.vector.tensor_mul(msq[:], mv[:, 0:1], mv[:, 0:1])
                ey2 = const.tile([96, 1], f32, name="ey2")
                nc.vector.tensor_add(ey2[:], mv[:, 1:2], msq[:])
                stats2 = const.tile([96, 2], f32, name="stats2")
                nc.vector.tensor_scalar_mul(stats2[:, 0:1], mv[:, 0:1], n_core)
                nc.vector.tensor_scalar_mul(stats2[:, 1:2], ey2[:], n_core)

                # --- AllReduce of [96,2] stats ------------------------------
                # explicit sem handshake: the CC engine's read of ib is NOT
                # tracked by Tile, so the CC must wait for the ib DMA's
                # transfer completion (else it reads stale DRAM -- a race
                # that shows up under trace-perturbed timing).
                ib = dram.tile([96, 2], f32, name="cc_in")
                ob = dram.tile([96, 2], f32, name="cc_out")
                gstats = const.tile([96, 2], f32, name="gstats")
                nc.gpsimd.dma_start(ib[:], stats2[:])
                nc.gpsimd.collective_compute(
                    "AllReduce",
                    bass.mybir.AluOpType.add,
                    replica_groups=[list(range(NCORES))],
                    ins=[ib.opt()],
                    outs=[ob.opt()],
                )
                nc.gpsimd.dma_start(gstats[:], ob[:])

            # scanbuf closed: hs/xr/ctl freed for the tail's xin prefetch
            ROWS = 8
            NSLAB = H // ROWS
            with (
                tc.tile_pool(name="finx", bufs=NSLAB) as finx,
                tc.tile_pool(name="fin", bufs=6) as fin,
            ):
                # prefetch all skip-input slabs; overlaps the AllReduce.
                # issue from sync/scalar so the gpsimd queue stays free for
                # the AllReduce sequence (its ib->CC->gstats ordering rides
                # on gpsimd program order; xins ahead of gstats would delay
                # the readback ~25us behind 7MB of prefetch traffic).
                xins = []
                for s in range(NSLAB):
                    lo = s * ROWS * W
                    xin = finx.tile([96, ROWS * W], bf16, name="xin")
                    eng = nc.sync if s % 2 == 0 else nc.scalar
                    eng.dma_start(xin[:], xbf_d[:, lo:lo + ROWS * W])
                    xins.append(xin)

                # global mean/var -> a = gamma*rsqrt(var+eps), b = beta - mu*a
                inv_n = 1.0 / (NCORES * HWTOT)
                mu_g = const.tile([96, 1], f32, name="mu_g")
                nc.vector.tensor_scalar_mul(mu_g[:], gstats[:, 0:1], inv_n)
                ey2_g = const.tile([96, 1], f32, name="ey2_g")
                nc.vector.tensor_scalar_mul(ey2_g[:], gstats[:, 1:2], inv_n)
                musq = const.tile([96, 1], f32, name="musq")
                nc.vector.tensor_mul(musq[:], mu_g[:], mu_g[:])
                var_g = const.tile([96, 1], f32, name="var_g")
                nc.vector.tensor_sub(var_g[:], ey2_g[:], musq[:])
                sd = const.tile([96, 1], f32, name="sd")
                nc.scalar.activation(sd[:], var_g[:], AF.Sqrt, bias=eps_s[:])
                rs = const.tile([96, 1], f32, name="rs")
                nc.vector.reciprocal(rs[:], sd[:])
                a_s = const.tile([96, 1], f32, name="a_s")
                nc.vector.tensor_mul(a_s[:], gb_s[:, 0:1], rs[:])
                nma = const.tile([96, 1], f32, name="nma")
                nc.vector.tensor_mul(nma[:], mu_g[:], a_s[:])
                b_s = const.tile([96, 1], f32, name="b_s")
                nc.vector.tensor_sub(b_s[:], gb_s[:, 1:2], nma[:])

                # relu(a*y+b) + x, write out (no matmuls in the tail)
                for s in range(NSLAB):
                    lo = s * ROWS * W
                    rtf = fin.tile([96, ROWS * W], f32, name="rtf")
                    ysl = y_s[:, s * ROWS:(s + 1) * ROWS, :]
                    nc.scalar.activation(
                        rtf[:], ysl.rearrange("p a b -> p (a b)"),
                        AF.Relu, bias=b_s[:], scale=a_s[:])
                    nc.vector.tensor_add(rtf[:], rtf[:], xins[s][:])
                    eng = (nc.sync, nc.scalar, nc.gpsimd)[s % 3]
                    eng.dma_start(out_d[:, lo:lo + ROWS * W], rtf[:])

    nc.finalize()
    return nc


def kernel(x, w_f, b_f, w_b, b_b, w_proj, gamma, beta):
    _install_ntff_hook()
    from concourse.bass_utils import run_bass_kernel_spmd

    x = np.asarray(x, np.float32)
    wx, wh, wp = _prep_weights(
        np.asarray(w_f, np.float32), np.asarray(b_f, np.float32),
        np.asarray(w_b, np.float32), np.asarray(b_b, np.float32),
        np.asarray(w_proj, np.float32),
    )
    gb = np.stack([np.asarray(gamma, np.float32),
                   np.asarray(beta, np.float32)], axis=1)  # [96, 2]

    if 'nc' not in _cached:
        _cached['nc'] = _build_program()
    nc = _cached['nc']

    in_maps = []
    for b in range(NCORES):
        xb = np.ascontiguousarray(x[b].reshape(C, HWTOT))
        in_maps.append({
            "xbf": xb.astype(ml_dtypes.bfloat16),
            "wx": wx,
            "wh": wh,
            "wp": wp,
            "gamma_beta": gb,
        })
    res = run_bass_kernel_spmd(nc, in_maps, list(range(NCORES)))
    if res.exec_time_ns is not None:
        print(f"HW exec time: {res.exec_time_ns} ns")
    out = np.stack([res.results[b]["out"].reshape(C, H, W)
                    for b in range(NCORES)], axis=0)
    return out.astype(np.float32)
